# revision 9
# baseline (speedup 1.0000x reference)
"""Single-head attention on 8 Trainium2 NeuronCores, batch-sharded.

Per core (one batch element b). v3: bf16 numerics (fp8 measured
error-infeasible: q/k-fp8 4e-2, P-fp8 1.4e-2 vs 2e-2 budget; and
DoubleRow gives no cycle-rate gain on this HW), with structural wins:

- No PE warmup: the clock ramp (HAM gate) rides the real projection
  matmuls. The old 44-matmul warmup delayed real work ~5us.
- Early DMA triggers are spread across engine queues (tensor/vector/
  scalar issue chunk-0 x halves + weights in parallel at ~0.6us each,
  sync takes chunks 1-3), so chunk-0 projections start earlier than a
  fully sync-serialized trigger chain would allow.
- Projections use N=512 (12 matmuls/chunk -> fewer issue overheads),
  one combined [Wv|Wq|Wk] weight tensor, one bias tensor (2 DMAs).
- Output is bf16 (denominator division on host in f32 -> error
  negligible), 5 output DMAs.

Layout (same trick as v1):
  A [Wv|Wq] -> psum; DVE bias-copy -> vq rows 0-63 v^T, 64-127 q^T.
  B [Wk] -> psum rows 64-127; kT rows 0-63 memset zero once.
  scores^T[kt] = kT[:,kt-cols]^T @ vq[:,q-cols]  (K=128; zero rows of
  kT annihilate the v rows of vq).

exp split ACT (exact, scale folded) / DVE (Schraudolph bf16 bit trick
on groups (qc,0),(qc,3) for qc 1-3; rel err of the mix ~0.7e-2).

PV (bf16): per k-tile matmul, M=65 (V plus ones row -> denominator
row), accumulated over 16 k-tiles into a 1-bank psum; PV(qc-1)
interleaves with scores(qc); PV(3) trails its own exps.

V layout: per-chunk PE transposes of vq rows into v65 [128, kt, 80]
+ DVE copy, one chunk late so DVE bias-adds are covered.

Epilogue per q-chunk: DVE copy psum->SBUF bf16, DMA out^T [65,512];
host divides by the denominator row and transposes.
"""

import numpy as np

VPITCH = 80

B, S, D, H = 8, 2048, 768, 64
DT = D // 128          # 6 d-tiles
NQ = S // 512          # 4 q-chunks of 512
NK = S // 128          # 16 k-tiles of 128
SCALE = (1.0 / np.sqrt(H)).item()
SCH_A = SCALE * np.log2(np.e).item() * 128.0   # Schraudolph slope (bf16)
SCH_C = 16248.5                                 # Schraudolph offset (tuned)
GROUPS = ((0, 3), (3, 6), (6, 9), (9, 12), (12, 15), (15, 16))
DVE_GROUPS = {(qc, g) for qc in (1, 2, 3) for g in (0, 3)}

_cache = {}


def _build():
    import concourse.mybir as mybir
    import concourse.tile as tile
    from concourse import bacc
    from concourse.masks import make_identity

    f32 = mybir.dt.float32
    bf16 = mybir.dt.bfloat16
    i16 = mybir.dt.int16
    Exp = mybir.ActivationFunctionType.Exp
    Mult = mybir.AluOpType.mult
    Add = mybir.AluOpType.add

    nc = bacc.Bacc(None)
    xp_d = nc.dram_tensor("xp", [128, NQ, DT, 512], bf16, kind="ExternalInput")
    wAB_d = nc.dram_tensor("wAB", [128, DT, 192], bf16, kind="ExternalInput")
    bAB_d = nc.dram_tensor("bAB", [128, 2], f32, kind="ExternalInput")
    out_d = nc.dram_tensor("out", [H + 1, NQ * 512], bf16, kind="ExternalOutput")

    with tile.TileContext(nc) as tc:
        with (
            tc.tile_pool(name="big", bufs=1) as big,
            tc.tile_pool(name="small", bufs=1) as small,
            tc.tile_pool(name="pt", bufs=8) as ptp,
            tc.tile_pool(name="res", bufs=2) as resp,
            tc.tile_pool(name="ps", bufs=2, space="PSUM") as ps,
        ):
            # ---- DMAs first: chunk-0 + weights fan out across engine
            # queues so their ~0.6us descriptor-gens run in parallel ----
            wAB = small.tile([128, DT, 192], bf16)
            bAB = small.tile([128, 2], f32)
            xT = big.tile([128, NQ, DT, 512], bf16)

            nc.sync.dma_start(out=xT[:, 0, 0:2], in_=xp_d[:, 0, 0:2, :])
            nc.scalar.dma_start(out=wAB, in_=wAB_d[:, :, :])
            nc.sync.dma_start(out=xT[:, 0, 2:6], in_=xp_d[:, 0, 2:6, :])
            nc.scalar.dma_start(out=bAB, in_=bAB_d[:, :])
            for c in (1, 2, 3):
                nc.sync.dma_start(out=xT[:, c], in_=xp_d[:, c, :, :])

            ident = small.tile([128, 128], f32)
            make_identity(nc, ident)
            identb = small.tile([128, 128], bf16)
            nc.gpsimd.tensor_copy(out=identb, in_=ident)

            # ---- persistent tensors ----
            vq = big.tile([128, S], bf16, tag="vq")   # v^T lo / q^T hi
            kT = big.tile([128, S], bf16, tag="kT")   # zeros lo / k^T hi
            nc.gpsimd.memset(kT[:H, :], 0.0)
            v65 = big.tile([128, NK, VPITCH], bf16, tag="v65")
            nc.gpsimd.memset(v65[:, :, H : H + 1], 1.0)

            alt = {"n": 0}

            def pj_tile(cols, nm, dtype=f32):
                tag = ("pj", "o")[alt["n"] % 2]
                alt["n"] += 1
                return ps.tile([128, cols], dtype, tag=tag, name=nm, bufs=1)

            pt_tiles = {}
            sc_state = {}
            outqs = [None] * NQ

            def emit_pv(qc, n):
                g = n // 3
                lo, _hi = GROUPS[g]
                pt = pt_tiles[(qc, g)]
                rhs_t = pt if pt.dtype == bf16 else pt.bitcast(bf16)
                slot = n - lo
                nc.tensor.matmul(
                    outqs[qc],
                    lhsT=v65[:, n, : H + 1],
                    rhs=rhs_t[:, slot * 512 : (slot + 1) * 512],
                    start=(n == 0),
                    stop=(n == NK - 1),
                )

            def emit_epilogue(qc, split=False):
                oT = resp.tile([H + 1, 512], bf16, tag="oT", name=f"oT{qc}")
                nc.vector.tensor_copy(out=oT, in_=outqs[qc])
                nc.gpsimd.dma_start(
                    out=out_d[:, qc * 512 : (qc + 1) * 512], in_=oT
                )

            def emit_score(qc, n):
                """scores^T for k-tile n against q-chunk qc (K=128 padded)."""
                st = sc_state.setdefault(qc, [None] * len(GROUPS))
                g = n // 3
                lo, hi = GROUPS[g]
                if st[g] is None:
                    st[g] = ps.tile(
                        [128, (hi - lo) * 512], f32, tag="sc", name=f"sc{qc}_{g}"
                    )
                slot = n - lo
                nc.tensor.matmul(
                    st[g][:, slot * 512 : (slot + 1) * 512],
                    lhsT=kT[:, n * 128 : (n + 1) * 128],
                    rhs=vq[:, qc * 512 : (qc + 1) * 512],
                    start=True,
                    stop=True,
                )
                if n == hi - 1:  # group full -> exp
                    cols = (hi - lo) * 512
                    if (qc, g) in DVE_GROUPS:
                        pt = ptp.tile([128, cols], i16, tag="pT", name=f"pt{qc}_{g}")
                        nc.vector.tensor_scalar(
                            out=pt,
                            in0=st[g],
                            scalar1=SCH_A,
                            scalar2=SCH_C,
                            op0=Mult,
                            op1=Add,
                        )
                    else:
                        pt = ptp.tile([128, cols], bf16, tag="pT", name=f"pt{qc}_{g}")
                        nc.scalar.activation(out=pt, in_=st[g], func=Exp, scale=SCALE)
                    pt_tiles[(qc, g)] = pt

            # ---- projection phase ----
            def chunk_tail(c):
                for j in range(4):
                    kt = 4 * c + j
                    tp = pj_tile(128, f"vtr{kt}", dtype=bf16)
                    nc.tensor.transpose(
                        tp[:, :H],
                        vq[:H, kt * 128 : (kt + 1) * 128],
                        identb[:H, :H],
                    )
                    nc.vector.tensor_copy(out=v65[:, kt, :H], in_=tp[:, :H])
                for j in range(4):
                    emit_score(0, 4 * c + j)

            for c in range(4):
                cc = slice(c * 512, (c + 1) * 512)

                def proj(mlo, mhi, wlo, whi, nm, c0=c):
                    p = pj_tile(512, f"ps{nm}{c0}")
                    for dt in range(DT):
                        nc.tensor.matmul(
                            p[mlo:mhi, :],
                            lhsT=wAB[:, dt, wlo:whi],
                            rhs=xT[:, c0, dt, :],
                            start=(dt == 0),
                            stop=(dt == DT - 1),
                        )
                    return p

                psA = proj(0, 128, 0, 128, "A")
                nc.vector.tensor_scalar_add(
                    out=vq[:, cc], in0=psA, scalar1=bAB[:, 0:1]
                )
                psB = proj(H, 128, 128, 192, "B")
                nc.vector.tensor_scalar_add(
                    out=kT[H:, cc], in0=psB[H:, :], scalar1=bAB[H:, 1:2]
                )
                if c >= 1:
                    chunk_tail(c - 1)
            chunk_tail(3)

            # ---- steady phase ----
            pv_cursor = [0] * NQ

            def pump_pv(qc, limit_n):
                while pv_cursor[qc] < min(limit_n, NK):
                    emit_pv(qc, pv_cursor[qc])
                    pv_cursor[qc] += 1

            outqs[0] = ps.tile([H + 1, 512], f32, tag="o", name="outq0", bufs=1)
            for qc in range(1, NQ):
                for n4 in range(0, NK, 4):
                    for n in range(n4, n4 + 4):
                        emit_score(qc, n)
                    pump_pv(qc - 1, n4 + 4)
                    if qc == NQ - 1:
                        ge = sum(1 for g in range(6) if (qc, g) in pt_tiles)
                        if ge >= 3:
                            if outqs[qc] is None:
                                outqs[qc] = ps.tile(
                                    [H + 1, 512],
                                    f32,
                                    tag=("o", "pj")[qc % 2],
                                    name=f"outq{qc}",
                                    bufs=1,
                                )
                            pump_pv(qc, 3 * (ge - 2))
                emit_epilogue(qc - 1)
                if qc < NQ - 1:
                    outqs[qc] = ps.tile(
                        [H + 1, 512],
                        f32,
                        tag=("o", "pj")[qc % 2],
                        name=f"outq{qc}",
                        bufs=1,
                    )
            pump_pv(NQ - 1, NK)
            emit_epilogue(NQ - 1, split=True)

    nc.compile()
    return nc


def _get_nc():
    if "nc" not in _cache:
        _cache["nc"] = _build()
    return _cache["nc"]


def _prep_inputs(x, Wq, bq, Wk, bk, Wv, bv):
    import ml_dtypes

    x = np.asarray(x, np.float32)
    Wq = np.asarray(Wq, np.float32)
    Wk = np.asarray(Wk, np.float32)
    Wv = np.asarray(Wv, np.float32)
    bq = np.asarray(bq, np.float32).ravel()
    bk = np.asarray(bk, np.float32).ravel()
    bv = np.asarray(bv, np.float32).ravel()

    # [768, 192] -> [128, DT, 192]; (p, dt, m) = w[dt*128+p, m]
    w = np.concatenate([Wv, Wq, Wk], axis=1)
    wAB = np.ascontiguousarray(
        w.reshape(DT, 128, 192).transpose(1, 0, 2)
    ).astype(ml_dtypes.bfloat16)
    bAB = np.ascontiguousarray(
        np.stack(
            [
                np.concatenate([bv, bq]),
                np.concatenate([np.zeros(H, np.float32), bk]),
            ],
            axis=1,
        )
    )
    common = {"wAB": wAB, "bAB": bAB}
    return x, common


def _xprep(xb):
    """[S, D] f32 -> [128, NQ, DT, 512]: (p, c, dt, j) = x[c*512+j, dt*128+p]"""
    import ml_dtypes

    t = xb.reshape(NQ, 512, DT, 128).transpose(3, 0, 2, 1)
    return np.ascontiguousarray(t).astype(ml_dtypes.bfloat16)


def _unshard_out(o):
    """[65, NQ*512] out^T bf16 with denominator row -> [S, H]"""
    o = np.asarray(o, np.float32)
    return (o[:H, :] / o[H : H + 1, :]).T


def _in_maps(x, common):
    return [{"xp": _xprep(x[b]), **common} for b in range(B)]


def kernel(x, Wq, bq, Wk, bk, Wv, bv, **_):
    from concourse.bass_utils import run_bass_kernel_spmd

    nc = _get_nc()
    x, common = _prep_inputs(x, Wq, bq, Wk, bk, Wv, bv)
    res = run_bass_kernel_spmd(nc, _in_maps(x, common), core_ids=list(range(B)))
    return np.stack([_unshard_out(res.results[b]["out"]) for b in range(B)])


# revision 10
# speedup vs baseline: 1.2291x; 1.2291x over previous
"""Single-head attention on 8 Trainium2 NeuronCores, batch-sharded.

Per core (one batch element b). v3: bf16 numerics (fp8 measured
error-infeasible: q/k-fp8 4e-2, P-fp8 1.4e-2 vs 2e-2 budget; and
DoubleRow gives no cycle-rate gain on this HW), with structural wins:

- No PE warmup: the clock ramp (HAM gate) rides the real projection
  matmuls. The old 44-matmul warmup delayed real work ~5us.
- Early DMA triggers are spread across engine queues (tensor/vector/
  scalar issue chunk-0 x halves + weights in parallel at ~0.6us each,
  sync takes chunks 1-3), so chunk-0 projections start earlier than a
  fully sync-serialized trigger chain would allow.
- Projections use N=512 (12 matmuls/chunk -> fewer issue overheads),
  one combined [Wv|Wq|Wk] weight tensor, one bias tensor (2 DMAs).
- Output is bf16 (denominator division on host in f32 -> error
  negligible), 5 output DMAs.

Layout (same trick as v1):
  A [Wv|Wq] -> psum; DVE bias-copy -> vq rows 0-63 v^T, 64-127 q^T.
  B [Wk] -> psum rows 64-127; kT rows 0-63 memset zero once.
  scores^T[kt] = kT[:,kt-cols]^T @ vq[:,q-cols]  (K=128; zero rows of
  kT annihilate the v rows of vq).

exp split ACT (exact, scale folded) / DVE (Schraudolph bf16 bit trick
on groups (qc,0),(qc,3) for qc 1-3; rel err of the mix ~0.7e-2).

PV (bf16): per k-tile matmul, M=65 (V plus ones row -> denominator
row), accumulated over 16 k-tiles into a 1-bank psum; PV(qc-1)
interleaves with scores(qc); PV(3) trails its own exps.

V layout: per-chunk PE transposes of vq rows into v65 [128, kt, 80]
+ DVE copy, one chunk late so DVE bias-adds are covered.

Epilogue per q-chunk: DVE copy psum->SBUF bf16, DMA out^T [65,512];
host divides by the denominator row and transposes.
"""

import numpy as np

VPITCH = 80

B, S, D, H = 8, 2048, 768, 64
DT = D // 128          # 6 d-tiles
NQ = S // 512          # 4 q-chunks of 512
NK = S // 128          # 16 k-tiles of 128
SCALE = (1.0 / np.sqrt(H)).item()
SCH_A = SCALE * np.log2(np.e).item() * 128.0   # Schraudolph slope (bf16)
SCH_C = 16248.5                                 # Schraudolph offset (tuned)
GROUPS = ((0, 3), (3, 6), (6, 9), (9, 12), (12, 15), (15, 16))
DVE_GROUPS = {(qc, g) for qc in (1, 2, 3) for g in (0, 3)}

_cache = {}


def _build():
    import concourse.mybir as mybir
    import concourse.tile as tile
    from concourse import bacc
    from concourse.masks import make_identity

    f32 = mybir.dt.float32
    bf16 = mybir.dt.bfloat16
    i16 = mybir.dt.int16
    Exp = mybir.ActivationFunctionType.Exp
    Mult = mybir.AluOpType.mult
    Add = mybir.AluOpType.add

    nc = bacc.Bacc(None)
    xp_d = nc.dram_tensor("xp", [128, NQ, DT, 512], bf16, kind="ExternalInput")
    wAB_d = nc.dram_tensor("wAB", [128, DT, 192], bf16, kind="ExternalInput")
    bAB_d = nc.dram_tensor("bAB", [128, 2], f32, kind="ExternalInput")
    out_d = nc.dram_tensor("out", [H + 1, NQ * 512], bf16, kind="ExternalOutput")

    with tile.TileContext(nc) as tc:
        with (
            tc.tile_pool(name="big", bufs=1) as big,
            tc.tile_pool(name="small", bufs=1) as small,
            tc.tile_pool(name="pt", bufs=8) as ptp,
            tc.tile_pool(name="res", bufs=2) as resp,
            tc.tile_pool(name="ps", bufs=2, space="PSUM") as ps,
        ):
            # ---- DMAs first: chunk-0 + weights fan out across engine
            # queues so their ~0.6us descriptor-gens run in parallel ----
            wAB = small.tile([128, DT, 192], bf16)
            bAB = small.tile([128, 2], f32)
            xT = big.tile([128, NQ, DT, 512], bf16)

            nc.sync.dma_start(out=xT[:, 0, 0:2], in_=xp_d[:, 0, 0:2, :])
            nc.scalar.dma_start(out=wAB, in_=wAB_d[:, :, :])
            nc.sync.dma_start(out=xT[:, 0, 2:6], in_=xp_d[:, 0, 2:6, :])
            nc.scalar.dma_start(out=bAB, in_=bAB_d[:, :])
            for c in (1, 2, 3):
                nc.sync.dma_start(out=xT[:, c], in_=xp_d[:, c, :, :])

            # PE warmup: the PE is data-starved until ~11.5us anyway
            # (DMA desc-gen + queue + sem latency), so dummy matmuls
            # complete the HAM clock ramp for free during the wait.
            wsrc = small.tile([128, 128], bf16)
            nc.gpsimd.memset(wsrc, 1.0)
            wps = ps.tile([128, 128], f32, tag="pj", name="warmps", bufs=1)
            for i in range(44):
                nc.tensor.matmul(wps, lhsT=wsrc, rhs=wsrc, start=True, stop=True)

            ident = small.tile([128, 128], f32)
            make_identity(nc, ident)
            identb = small.tile([128, 128], bf16)
            nc.gpsimd.tensor_copy(out=identb, in_=ident)

            # ---- persistent tensors ----
            vq = big.tile([128, S], bf16, tag="vq")   # v^T lo / q^T hi
            kT = big.tile([128, S], bf16, tag="kT")   # zeros lo / k^T hi
            nc.gpsimd.memset(kT[:H, :], 0.0)
            v65 = big.tile([128, NK, VPITCH], bf16, tag="v65")
            nc.gpsimd.memset(v65[:, :, H : H + 1], 1.0)

            alt = {"n": 0}

            def pj_tile(cols, nm, dtype=f32):
                tag = ("pj", "o")[alt["n"] % 2]
                alt["n"] += 1
                return ps.tile([128, cols], dtype, tag=tag, name=nm, bufs=1)

            pt_tiles = {}
            sc_state = {}
            outqs = [None] * NQ

            def emit_pv(qc, n):
                g = n // 3
                lo, _hi = GROUPS[g]
                pt = pt_tiles[(qc, g)]
                rhs_t = pt if pt.dtype == bf16 else pt.bitcast(bf16)
                slot = n - lo
                nc.tensor.matmul(
                    outqs[qc],
                    lhsT=v65[:, n, : H + 1],
                    rhs=rhs_t[:, slot * 512 : (slot + 1) * 512],
                    start=(n == 0),
                    stop=(n == NK - 1),
                )

            def emit_epilogue(qc, split=False):
                oT = resp.tile([H + 1, 512], bf16, tag="oT", name=f"oT{qc}")
                if split:  # final epilogue: parallel desc-gen on 2 queues
                    for hh, eng in ((0, nc.sync), (1, nc.scalar)):
                        nc.vector.tensor_copy(
                            out=oT[:, hh * 256 : (hh + 1) * 256],
                            in_=outqs[qc][:, hh * 256 : (hh + 1) * 256],
                        )
                        eng.dma_start(
                            out=out_d[
                                :, qc * 512 + hh * 256 : qc * 512 + (hh + 1) * 256
                            ],
                            in_=oT[:, hh * 256 : (hh + 1) * 256],
                        )
                else:
                    nc.vector.tensor_copy(out=oT, in_=outqs[qc])
                    nc.sync.dma_start(
                        out=out_d[:, qc * 512 : (qc + 1) * 512], in_=oT
                    )

            def emit_score(qc, n):
                """scores^T for k-tile n against q-chunk qc (K=128 padded)."""
                st = sc_state.setdefault(qc, [None] * len(GROUPS))
                g = n // 3
                lo, hi = GROUPS[g]
                if st[g] is None:
                    st[g] = ps.tile(
                        [128, (hi - lo) * 512], f32, tag="sc", name=f"sc{qc}_{g}"
                    )
                slot = n - lo
                nc.tensor.matmul(
                    st[g][:, slot * 512 : (slot + 1) * 512],
                    lhsT=kT[:, n * 128 : (n + 1) * 128],
                    rhs=vq[:, qc * 512 : (qc + 1) * 512],
                    start=True,
                    stop=True,
                )
                if n == hi - 1:  # group full -> exp
                    cols = (hi - lo) * 512
                    if (qc, g) in DVE_GROUPS:
                        pt = ptp.tile([128, cols], i16, tag="pT", name=f"pt{qc}_{g}")
                        nc.vector.tensor_scalar(
                            out=pt,
                            in0=st[g],
                            scalar1=SCH_A,
                            scalar2=SCH_C,
                            op0=Mult,
                            op1=Add,
                        )
                    else:
                        pt = ptp.tile([128, cols], bf16, tag="pT", name=f"pt{qc}_{g}")
                        nc.scalar.activation(out=pt, in_=st[g], func=Exp, scale=SCALE)
                    pt_tiles[(qc, g)] = pt

            # ---- projection phase ----
            def chunk_tail(c):
                for j in range(4):
                    kt = 4 * c + j
                    tp = pj_tile(128, f"vtr{kt}", dtype=bf16)
                    nc.tensor.transpose(
                        tp[:, :H],
                        vq[:H, kt * 128 : (kt + 1) * 128],
                        identb[:H, :H],
                    )
                    nc.vector.tensor_copy(out=v65[:, kt, :H], in_=tp[:, :H])
                for j in range(4):
                    emit_score(0, 4 * c + j)

            for c in range(4):
                cc = slice(c * 512, (c + 1) * 512)

                def proj(mlo, mhi, wlo, whi, nm, c0=c):
                    p = pj_tile(512, f"ps{nm}{c0}")
                    for dt in range(DT):
                        nc.tensor.matmul(
                            p[mlo:mhi, :],
                            lhsT=wAB[:, dt, wlo:whi],
                            rhs=xT[:, c0, dt, :],
                            start=(dt == 0),
                            stop=(dt == DT - 1),
                        )
                    return p

                psA = proj(0, 128, 0, 128, "A")
                nc.vector.tensor_scalar_add(
                    out=vq[:, cc], in0=psA, scalar1=bAB[:, 0:1]
                )
                psB = proj(H, 128, 128, 192, "B")
                nc.vector.tensor_scalar_add(
                    out=kT[H:, cc], in0=psB[H:, :], scalar1=bAB[H:, 1:2]
                )
                if c >= 1:
                    chunk_tail(c - 1)
            chunk_tail(3)

            # ---- steady phase ----
            pv_cursor = [0] * NQ

            def pump_pv(qc, limit_n):
                while pv_cursor[qc] < min(limit_n, NK):
                    emit_pv(qc, pv_cursor[qc])
                    pv_cursor[qc] += 1

            outqs[0] = ps.tile([H + 1, 512], f32, tag="o", name="outq0", bufs=1)
            for qc in range(1, NQ):
                for n4 in range(0, NK, 4):
                    for n in range(n4, n4 + 4):
                        emit_score(qc, n)
                    pump_pv(qc - 1, n4 + 4)
                    if qc == NQ - 1:
                        ge = sum(1 for g in range(6) if (qc, g) in pt_tiles)
                        if ge >= 3:
                            if outqs[qc] is None:
                                outqs[qc] = ps.tile(
                                    [H + 1, 512],
                                    f32,
                                    tag=("o", "pj")[qc % 2],
                                    name=f"outq{qc}",
                                    bufs=1,
                                )
                            pump_pv(qc, 3 * (ge - 2))
                emit_epilogue(qc - 1)
                if qc < NQ - 1:
                    outqs[qc] = ps.tile(
                        [H + 1, 512],
                        f32,
                        tag=("o", "pj")[qc % 2],
                        name=f"outq{qc}",
                        bufs=1,
                    )
            pump_pv(NQ - 1, NK)
            emit_epilogue(NQ - 1, split=True)

    nc.compile()
    return nc


def _get_nc():
    if "nc" not in _cache:
        _cache["nc"] = _build()
    return _cache["nc"]


def _prep_inputs(x, Wq, bq, Wk, bk, Wv, bv):
    import ml_dtypes

    x = np.asarray(x, np.float32)
    Wq = np.asarray(Wq, np.float32)
    Wk = np.asarray(Wk, np.float32)
    Wv = np.asarray(Wv, np.float32)
    bq = np.asarray(bq, np.float32).ravel()
    bk = np.asarray(bk, np.float32).ravel()
    bv = np.asarray(bv, np.float32).ravel()

    # [768, 192] -> [128, DT, 192]; (p, dt, m) = w[dt*128+p, m]
    w = np.concatenate([Wv, Wq, Wk], axis=1)
    wAB = np.ascontiguousarray(
        w.reshape(DT, 128, 192).transpose(1, 0, 2)
    ).astype(ml_dtypes.bfloat16)
    bAB = np.ascontiguousarray(
        np.stack(
            [
                np.concatenate([bv, bq]),
                np.concatenate([np.zeros(H, np.float32), bk]),
            ],
            axis=1,
        )
    )
    common = {"wAB": wAB, "bAB": bAB}
    return x, common


def _xprep(xb):
    """[S, D] f32 -> [128, NQ, DT, 512]: (p, c, dt, j) = x[c*512+j, dt*128+p]"""
    import ml_dtypes

    t = xb.reshape(NQ, 512, DT, 128).transpose(3, 0, 2, 1)
    return np.ascontiguousarray(t).astype(ml_dtypes.bfloat16)


def _unshard_out(o):
    """[65, NQ*512] out^T bf16 with denominator row -> [S, H]"""
    o = np.asarray(o, np.float32)
    return (o[:H, :] / o[H : H + 1, :]).T


def _in_maps(x, common):
    return [{"xp": _xprep(x[b]), **common} for b in range(B)]


def kernel(x, Wq, bq, Wk, bk, Wv, bv, **_):
    from concourse.bass_utils import run_bass_kernel_spmd

    nc = _get_nc()
    x, common = _prep_inputs(x, Wq, bq, Wk, bk, Wv, bv)
    res = run_bass_kernel_spmd(nc, _in_maps(x, common), core_ids=list(range(B)))
    return np.stack([_unshard_out(res.results[b]["out"]) for b in range(B)])


# revision 11
# speedup vs baseline: 1.2440x; 1.0121x over previous
"""Single-head attention on 8 Trainium2 NeuronCores, batch-sharded.

Per core (one batch element b). v3: bf16 numerics (fp8 measured
error-infeasible: q/k-fp8 4e-2, P-fp8 1.4e-2 vs 2e-2 budget; and
DoubleRow gives no cycle-rate gain on this HW), with structural wins:

- No PE warmup: the clock ramp (HAM gate) rides the real projection
  matmuls. The old 44-matmul warmup delayed real work ~5us.
- Early DMA triggers are spread across engine queues (tensor/vector/
  scalar issue chunk-0 x halves + weights in parallel at ~0.6us each,
  sync takes chunks 1-3), so chunk-0 projections start earlier than a
  fully sync-serialized trigger chain would allow.
- Projections use N=512 (12 matmuls/chunk -> fewer issue overheads),
  one combined [Wv|Wq|Wk] weight tensor, one bias tensor (2 DMAs).
- Output is bf16 (denominator division on host in f32 -> error
  negligible), 5 output DMAs.

Layout (same trick as v1):
  A [Wv|Wq] -> psum; DVE bias-copy -> vq rows 0-63 v^T, 64-127 q^T.
  B [Wk] -> psum rows 64-127; kT rows 0-63 memset zero once.
  scores^T[kt] = kT[:,kt-cols]^T @ vq[:,q-cols]  (K=128; zero rows of
  kT annihilate the v rows of vq).

exp split ACT (exact, scale folded) / DVE (Schraudolph bf16 bit trick
on groups (qc,0),(qc,3) for qc 1-3; rel err of the mix ~0.7e-2).

PV (bf16): per k-tile matmul, M=65 (V plus ones row -> denominator
row), accumulated over 16 k-tiles into a 1-bank psum; PV(qc-1)
interleaves with scores(qc); PV(3) trails its own exps.

V layout: per-chunk PE transposes of vq rows into v65 [128, kt, 80]
+ DVE copy, one chunk late so DVE bias-adds are covered.

Epilogue per q-chunk: DVE copy psum->SBUF bf16, DMA out^T [65,512];
host divides by the denominator row and transposes.
"""

import numpy as np

VPITCH = 80

B, S, D, H = 8, 2048, 768, 64
DT = D // 128          # 6 d-tiles
NQ = S // 512          # 4 q-chunks of 512
NK = S // 128          # 16 k-tiles of 128
SCALE = (1.0 / np.sqrt(H)).item()
SCH_A = SCALE * np.log2(np.e).item() * 128.0   # Schraudolph slope (bf16)
SCH_C = 16248.5                                 # Schraudolph offset (tuned)
GROUPS = tuple((2 * i, 2 * i + 2) for i in range(8))
DVE_GROUPS = {(qc, g) for qc in (1, 2, 3) for g in (0, 4)}

_cache = {}


def _build():
    import concourse.mybir as mybir
    import concourse.tile as tile
    from concourse import bacc
    from concourse.masks import make_identity

    f32 = mybir.dt.float32
    bf16 = mybir.dt.bfloat16
    i16 = mybir.dt.int16
    Exp = mybir.ActivationFunctionType.Exp
    Mult = mybir.AluOpType.mult
    Add = mybir.AluOpType.add

    nc = bacc.Bacc(None)
    xp_d = nc.dram_tensor("xp", [128, NQ, DT, 512], bf16, kind="ExternalInput")
    wAB_d = nc.dram_tensor("wAB", [128, DT, 192], bf16, kind="ExternalInput")
    bAB_d = nc.dram_tensor("bAB", [128, 2], f32, kind="ExternalInput")
    out_d = nc.dram_tensor("out", [H + 1, NQ * 512], bf16, kind="ExternalOutput")

    with tile.TileContext(nc) as tc:
        with (
            tc.tile_pool(name="big", bufs=1) as big,
            tc.tile_pool(name="small", bufs=1) as small,
            tc.tile_pool(name="pt", bufs=8) as ptp,
            tc.tile_pool(name="res", bufs=2) as resp,
            tc.tile_pool(name="ps", bufs=2, space="PSUM") as ps,
        ):
            # ---- DMAs first: chunk-0 + weights fan out across engine
            # queues so their ~0.6us descriptor-gens run in parallel ----
            wAB = small.tile([128, DT, 192], bf16)
            bAB = small.tile([128, 2], f32)
            xT = big.tile([128, NQ, DT, 512], bf16)

            nc.sync.dma_start(out=xT[:, 0, 0:2], in_=xp_d[:, 0, 0:2, :])
            nc.scalar.dma_start(out=wAB, in_=wAB_d[:, :, :])
            nc.sync.dma_start(out=xT[:, 0, 2:6], in_=xp_d[:, 0, 2:6, :])
            nc.scalar.dma_start(out=bAB, in_=bAB_d[:, :])
            for c in (1, 2, 3):
                nc.sync.dma_start(out=xT[:, c], in_=xp_d[:, c, :, :])

            # PE warmup: the PE is data-starved until ~11.5us anyway
            # (DMA desc-gen + queue + sem latency), so dummy matmuls
            # complete the HAM clock ramp for free during the wait.
            wsrc = small.tile([128, 128], bf16)
            nc.gpsimd.memset(wsrc, 1.0)
            wps = ps.tile([128, 128], f32, tag="pj", name="warmps", bufs=1)
            for i in range(44):
                nc.tensor.matmul(wps, lhsT=wsrc, rhs=wsrc, start=True, stop=True)

            ident = small.tile([128, 128], f32)
            make_identity(nc, ident)
            identb = small.tile([128, 128], bf16)
            nc.gpsimd.tensor_copy(out=identb, in_=ident)

            # ---- persistent tensors ----
            vq = big.tile([128, S], bf16, tag="vq")   # v^T lo / q^T hi
            kT = big.tile([128, S], bf16, tag="kT")   # zeros lo / k^T hi
            nc.gpsimd.memset(kT[:H, :], 0.0)
            v65 = big.tile([128, NK, VPITCH], bf16, tag="v65")
            nc.gpsimd.memset(v65[:, :, H : H + 1], 1.0)

            alt = {"n": 0}

            def pj_tile(cols, nm, dtype=f32):
                tag = ("pj", "o")[alt["n"] % 2]
                alt["n"] += 1
                return ps.tile([128, cols], dtype, tag=tag, name=nm, bufs=1)

            pt_tiles = {}
            sc_state = {}
            outqs = [None] * NQ

            def emit_pv(qc, n):
                g = n // 2
                lo, _hi = GROUPS[g]
                pt = pt_tiles[(qc, g)]
                rhs_t = pt if pt.dtype == bf16 else pt.bitcast(bf16)
                slot = n - lo
                nc.tensor.matmul(
                    outqs[qc],
                    lhsT=v65[:, n, : H + 1],
                    rhs=rhs_t[:, slot * 512 : (slot + 1) * 512],
                    start=(n == 0),
                    stop=(n == NK - 1),
                )

            def emit_epilogue(qc, split=False):
                oT = resp.tile([H + 1, 512], bf16, tag="oT", name=f"oT{qc}")
                if split:  # final epilogue: parallel desc-gen on 2 queues
                    for hh, eng in ((0, nc.sync), (1, nc.scalar)):
                        nc.vector.tensor_copy(
                            out=oT[:, hh * 256 : (hh + 1) * 256],
                            in_=outqs[qc][:, hh * 256 : (hh + 1) * 256],
                        )
                        eng.dma_start(
                            out=out_d[
                                :, qc * 512 + hh * 256 : qc * 512 + (hh + 1) * 256
                            ],
                            in_=oT[:, hh * 256 : (hh + 1) * 256],
                        )
                else:
                    nc.vector.tensor_copy(out=oT, in_=outqs[qc])
                    nc.sync.dma_start(
                        out=out_d[:, qc * 512 : (qc + 1) * 512], in_=oT
                    )

            def emit_score(qc, n):
                """scores^T for k-tile n against q-chunk qc (K=128 padded)."""
                st = sc_state.setdefault(qc, [None] * len(GROUPS))
                g = n // 2
                lo, hi = GROUPS[g]
                if st[g] is None:
                    st[g] = ps.tile(
                        [128, (hi - lo) * 512], f32, tag="sc", name=f"sc{qc}_{g}",
                        bufs=3,
                    )
                slot = n - lo
                nc.tensor.matmul(
                    st[g][:, slot * 512 : (slot + 1) * 512],
                    lhsT=kT[:, n * 128 : (n + 1) * 128],
                    rhs=vq[:, qc * 512 : (qc + 1) * 512],
                    start=True,
                    stop=True,
                )
                if n == hi - 1:  # group full -> exp
                    cols = (hi - lo) * 512
                    if (qc, g) in DVE_GROUPS:
                        pt = ptp.tile([128, cols], i16, tag="pT", name=f"pt{qc}_{g}")
                        nc.vector.tensor_scalar(
                            out=pt,
                            in0=st[g],
                            scalar1=SCH_A,
                            scalar2=SCH_C,
                            op0=Mult,
                            op1=Add,
                        )
                    else:
                        pt = ptp.tile([128, cols], bf16, tag="pT", name=f"pt{qc}_{g}")
                        nc.scalar.activation(out=pt, in_=st[g], func=Exp, scale=SCALE)
                    pt_tiles[(qc, g)] = pt

            # ---- projection phase ----
            def chunk_tail(c):
                for j in range(4):
                    kt = 4 * c + j
                    tp = pj_tile(128, f"vtr{kt}", dtype=bf16)
                    nc.tensor.transpose(
                        tp[:, :H],
                        vq[:H, kt * 128 : (kt + 1) * 128],
                        identb[:H, :H],
                    )
                    nc.vector.tensor_copy(out=v65[:, kt, :H], in_=tp[:, :H])
                for j in range(4):
                    emit_score(0, 4 * c + j)

            for c in range(4):
                cc = slice(c * 512, (c + 1) * 512)

                def proj(mlo, mhi, wlo, whi, nm, c0=c):
                    p = pj_tile(512, f"ps{nm}{c0}")
                    for dt in range(DT):
                        nc.tensor.matmul(
                            p[mlo:mhi, :],
                            lhsT=wAB[:, dt, wlo:whi],
                            rhs=xT[:, c0, dt, :],
                            start=(dt == 0),
                            stop=(dt == DT - 1),
                        )
                    return p

                psA = proj(0, 128, 0, 128, "A")
                nc.vector.tensor_scalar_add(
                    out=vq[:, cc], in0=psA, scalar1=bAB[:, 0:1]
                )
                psB = proj(H, 128, 128, 192, "B")
                nc.vector.tensor_scalar_add(
                    out=kT[H:, cc], in0=psB[H:, :], scalar1=bAB[H:, 1:2]
                )
                if c >= 1:
                    chunk_tail(c - 1)
            chunk_tail(3)

            # ---- steady phase ----
            pv_cursor = [0] * NQ

            def pump_pv(qc, limit_n):
                while pv_cursor[qc] < min(limit_n, NK):
                    emit_pv(qc, pv_cursor[qc])
                    pv_cursor[qc] += 1

            outqs[0] = ps.tile([H + 1, 512], f32, tag="o", name="outq0", bufs=1)
            for qc in range(1, NQ):
                for n4 in range(0, NK, 4):
                    for n in range(n4, n4 + 4):
                        emit_score(qc, n)
                    pump_pv(qc - 1, n4 + 4)
                    if qc == NQ - 1:
                        ge = sum(1 for g in range(len(GROUPS)) if (qc, g) in pt_tiles)
                        if ge >= 3:
                            if outqs[qc] is None:
                                outqs[qc] = ps.tile(
                                    [H + 1, 512],
                                    f32,
                                    tag=("o", "pj")[qc % 2],
                                    name=f"outq{qc}",
                                    bufs=1,
                                )
                            pump_pv(qc, 2 * (ge - 2))
                emit_epilogue(qc - 1)
                if qc < NQ - 1:
                    outqs[qc] = ps.tile(
                        [H + 1, 512],
                        f32,
                        tag=("o", "pj")[qc % 2],
                        name=f"outq{qc}",
                        bufs=1,
                    )
            pump_pv(NQ - 1, NK)
            emit_epilogue(NQ - 1, split=True)

    nc.compile()
    return nc


def _get_nc():
    if "nc" not in _cache:
        _cache["nc"] = _build()
    return _cache["nc"]


def _prep_inputs(x, Wq, bq, Wk, bk, Wv, bv):
    import ml_dtypes

    x = np.asarray(x, np.float32)
    Wq = np.asarray(Wq, np.float32)
    Wk = np.asarray(Wk, np.float32)
    Wv = np.asarray(Wv, np.float32)
    bq = np.asarray(bq, np.float32).ravel()
    bk = np.asarray(bk, np.float32).ravel()
    bv = np.asarray(bv, np.float32).ravel()

    # [768, 192] -> [128, DT, 192]; (p, dt, m) = w[dt*128+p, m]
    w = np.concatenate([Wv, Wq, Wk], axis=1)
    wAB = np.ascontiguousarray(
        w.reshape(DT, 128, 192).transpose(1, 0, 2)
    ).astype(ml_dtypes.bfloat16)
    bAB = np.ascontiguousarray(
        np.stack(
            [
                np.concatenate([bv, bq]),
                np.concatenate([np.zeros(H, np.float32), bk]),
            ],
            axis=1,
        )
    )
    common = {"wAB": wAB, "bAB": bAB}
    return x, common


def _xprep(xb):
    """[S, D] f32 -> [128, NQ, DT, 512]: (p, c, dt, j) = x[c*512+j, dt*128+p]"""
    import ml_dtypes

    t = xb.reshape(NQ, 512, DT, 128).transpose(3, 0, 2, 1)
    return np.ascontiguousarray(t).astype(ml_dtypes.bfloat16)


def _unshard_out(o):
    """[65, NQ*512] out^T bf16 with denominator row -> [S, H]"""
    o = np.asarray(o, np.float32)
    return (o[:H, :] / o[H : H + 1, :]).T


def _in_maps(x, common):
    return [{"xp": _xprep(x[b]), **common} for b in range(B)]


def kernel(x, Wq, bq, Wk, bk, Wv, bv, **_):
    from concourse.bass_utils import run_bass_kernel_spmd

    nc = _get_nc()
    x, common = _prep_inputs(x, Wq, bq, Wk, bk, Wv, bv)
    res = run_bass_kernel_spmd(nc, _in_maps(x, common), core_ids=list(range(B)))
    return np.stack([_unshard_out(res.results[b]["out"]) for b in range(B)])
